# revision 1
# baseline (speedup 1.0000x reference)
"""GAT spatio-temporal model Trainium2 kernel (v4).

Sharding: data-parallel over batch B=8 -> 8 NeuronCores (1 graph each).

v4 core trick: exp(leaky_relu(s1[n]+s2[m])) = max(E1*E2, E1a*E2a) with
E = exp(s), Ea = exp(alpha*s) (exp monotone, lrelu(x) = max(x, a*x)).
Factor p = E1a[n] * E2[m] * max(E1b[n], E2inv[m]) with E1b = exp((1-a)s1),
E2inv = exp(-(1-a)s2).  E1a[n] is constant along the softmax axis (m) and
cancels; E2[m] folds into the den / AV matmul lhsT weights.  The whole
[N,N] attention tensor is then ONE fused DVE op per 128-chunk:
scalar_tensor_tensor(out, E1b_bcast, E2inv_col, maskT, max, mult).
No N^2 ScalarE work; s1/s2 come from one matmul via precomposed W@a.
All N^2 matmuls bf16; LN matmuls float32r.

Shapes (hardcoded): B=8, N=512, Din=64, H=8, F=128, L=2.
"""
import os
import numpy as np
from contextlib import ExitStack

import concourse.bass as bass
import concourse.tile as tile
from concourse import bacc, mybir
from concourse.bass_utils import run_bass_kernel_spmd
from concourse.masks import make_identity

F32 = mybir.dt.float32
F32R = mybir.dt.float32r
BF16 = mybir.dt.bfloat16
AF = mybir.ActivationFunctionType
OP = mybir.AluOpType

B, N, DIN, H, F, L = 8, 512, 64, 8, 128, 2
NCHUNK = N // 128  # 4
ALPHA = 0.2
BETA = 1.0 - ALPHA
LN_EPS = 1e-5

# gpsimd compute offload is unavailable in this toolchain (no Pool
# lowering pass); all elementwise work stays on Vector/Scalar.
GP_STT = 0
GP_ELU = False
GP_EMAX = False
GP_PSC = 0

_CACHE = {}


def _bcast_row(ap_row):
    return bass.AP(tensor=ap_row.tensor, offset=ap_row.offset, ap=[[0, 128], [1, N]])


def _r(ap):
    return ap.bitcast(F32R)


def build_nc():
    nc = bacc.Bacc("TRN2", target_bir_lowering=False, debug=False)

    x_d = nc.dram_tensor("x", [N, DIN], F32, kind="ExternalInput").ap()
    adj_d = nc.dram_tensor("adj", [N, N], mybir.dt.int32, kind="ExternalInput").ap()
    Wp_d = nc.dram_tensor("Wp", [DIN, F], F32, kind="ExternalInput").ap()
    bp_d = nc.dram_tensor("bp", [F], F32, kind="ExternalInput").ap()
    Wh_d = nc.dram_tensor("W_heads", [L, H, F, F], F32, kind="ExternalInput").ap()
    ah_d = nc.dram_tensor("a_heads", [L, H, 2 * F], F32, kind="ExternalInput").ap()
    Wo_d = nc.dram_tensor("W_out", [L, H * F, F], F32, kind="ExternalInput").ap()
    ao_d = nc.dram_tensor("a_out", [L, 2 * F], F32, kind="ExternalInput").ap()
    g_d = nc.dram_tensor("ln_g", [L, F], F32, kind="ExternalInput").ap()
    b_d = nc.dram_tensor("ln_b", [L, F], F32, kind="ExternalInput").ap()
    out_d = nc.dram_tensor("out", [N, F], F32, kind="ExternalOutput").ap()
    # DRAM bounce buffers: E1b rows (per layer) + per-head recip rows
    ebl_d = [nc.dram_tensor(f"eblk{l}", [16, N], BF16, kind="ExternalOutput").ap()
             for l in range(L)]
    scr_d = [nc.dram_tensor(f"scratch{i}", [1, N], BF16, kind="ExternalOutput").ap()
             for i in range(18)]

    with tile.TileContext(nc) as tc, ExitStack() as ctx:
        const = ctx.enter_context(tc.tile_pool(name="const", bufs=1))
        sx = ctx.enter_context(tc.tile_pool(name="sx", bufs=2))
        sproj = ctx.enter_context(tc.tile_pool(name="sproj", bufs=10))
        sbcast = ctx.enter_context(tc.tile_pool(name="sbcast", bufs=10))
        sexp = ctx.enter_context(tc.tile_pool(name="sexp", bufs=4))
        smulti = ctx.enter_context(tc.tile_pool(name="smulti", bufs=9))
        sbig = ctx.enter_context(tc.tile_pool(name="sbig", bufs=3))
        srow = ctx.enter_context(tc.tile_pool(name="srow", bufs=5))
        shd = ctx.enter_context(tc.tile_pool(name="shd", bufs=4))
        smask = ctx.enter_context(tc.tile_pool(name="smask", bufs=4))
        pou = ctx.enter_context(tc.tile_pool(name="pou", bufs=3, space="PSUM"))
        pmisc = ctx.enter_context(tc.tile_pool(name="pmisc", bufs=2, space="PSUM"))
        prow = ctx.enter_context(tc.tile_pool(name="prow", bufs=3, space="PSUM"))

        # ---------------- constants ----------------
        ones_row = const.tile([1, N], F32)
        nc.vector.memset(ones_row, 1.0)
        ones_row_bf = const.tile([1, N], BF16)
        nc.vector.memset(ones_row_bf, 1.0)
        ones_col = const.tile([128, 1], F32)
        nc.vector.memset(ones_col, 1.0)
        ones_col_bf = const.tile([128, 1], BF16)
        nc.vector.memset(ones_col_bf, 1.0)
        ident = const.tile([128, 128], F32)
        make_identity(nc, ident)
        ident_bf = const.tile([128, 128], BF16)
        nc.vector.tensor_copy(ident_bf, ident)
        eps1 = const.tile([1, 1], F32)
        nc.vector.memset(eps1, LN_EPS)

        Wp_sb = const.tile([DIN, F], BF16)
        nc.gpsimd.dma_start(Wp_sb, Wp_d)
        bp_col = const.tile([F, 1], F32)
        nc.sync.dma_start(bp_col, bp_d.rearrange("(f one) -> f one", one=1))
        x_chunks = []
        for c in range(NCHUNK):
            xc = shd.tile([128, DIN], F32, tag="xchunk")
            nc.sync.dma_start(xc, x_d[bass.ts(c, 128), :])
            x_chunks.append(xc)

        # layer-0 head weights on fast HW queue (fp32) + DVE cast; the rest
        # trickle in on the gpsimd software queue with cast
        ah_ball = const.tile([F, L * H, 2], BF16)
        nc.gpsimd.dma_start(ah_ball, ah_d.rearrange("l h (t f) -> f (l h) t", t=2))
        ah_bf = [[ah_ball[:, l * H + h, :] for h in range(H)] for l in range(L)]

        Wh_ball = [const.tile([F, H, F], BF16, name=f"WhB{l}") for l in range(L)]
        Wh0_f = const.tile([F, H, F], F32)
        nc.sync.dma_start(Wh0_f, Wh_d[0].rearrange("h i o -> i h o"))
        nc.vector.tensor_copy(Wh_ball[0], Wh0_f)
        nc.gpsimd.dma_start(Wh_ball[1], Wh_d[1].rearrange("h i o -> i h o"))
        Wh_bf = [[Wh_ball[l][:, h, :] for h in range(H)] for l in range(L)]

        Wo_ball = [const.tile([128, H, F], BF16, name=f"WoB{l}") for l in range(L)]
        for l in range(L):
            nc.gpsimd.dma_start(Wo_ball[l], Wo_d[l].rearrange("(c p) f -> p c f", p=128))
        Wo_bf = Wo_ball

        ao_ball = const.tile([F, L, 2], BF16)
        nc.gpsimd.dma_start(ao_ball, ao_d.rearrange("l (t f) -> f l t", t=2))
        ao_bf = [ao_ball[:, l, :] for l in range(L)]

        g_all = const.tile([1, L, F], F32)
        nc.scalar.dma_start(g_all, g_d.rearrange("l f -> (l f)").rearrange(
            "(one l f) -> one l f", one=1, l=L))
        b_all = const.tile([1, L, F], F32)
        nc.scalar.dma_start(b_all, b_d.rearrange("l f -> (l f)").rearrange(
            "(one l f) -> one l f", one=1, l=L))
        gc_all = const.tile([F, L], F32)
        nc.scalar.dma_start(gc_all, g_d.rearrange("l f -> f l"))
        g_row = [g_all[:, l, :] for l in range(L)]
        b_row = [b_all[:, l, :] for l in range(L)]
        g_col = [gc_all[:, l:l + 1] for l in range(L)]
        gb_bf = const.tile([1, 2 * L, F], BF16)
        nc.vector.tensor_copy(gb_bf[:, 0:L, :], g_all)
        nc.vector.tensor_copy(gb_bf[:, L:2 * L, :], b_all)
        g_row_bf = [gb_bf[:, l, :] for l in range(L)]
        b_row_bf = [gb_bf[:, L + l, :] for l in range(L)]

        # ------------- WhT (transposed head weights) + Wtilde = W @ a -------
        WhT_ball = [const.tile([F, H, F], BF16, name=f"WhT{l}") for l in range(L)]
        for l in range(L):
            for h in range(H):
                pt = pou.tile([128, 128], BF16, tag="oU")
                nc.tensor.transpose(pt, Wh_bf[l][h], ident_bf)
                if h % 2 == 0:
                    nc.scalar.activation(WhT_ball[l][:, h, :], pt, AF.Copy)
                else:
                    nc.vector.tensor_copy(WhT_ball[l][:, h, :], pt)
        Wt_bf = [const.tile([F, 2 * H], BF16, name=f"Wt{l}") for l in range(L)]
        for l in range(L):
            pw = prow.tile([128, 2 * H], F32, tag="prow")
            for h in range(H):
                nc.tensor.matmul(pw[:, 2 * h:2 * h + 2], WhT_ball[l][:, h, :],
                                 ah_bf[l][h], start=True, stop=True)
            nc.scalar.activation(Wt_bf[l], pw, AF.Copy)

        # ---------------- x -> xT, input projection ----------------
        xT = const.tile([DIN, N], BF16)
        ph = pmisc.tile([128, N], F32, tag="pbig")
        hT = sbig.tile([128, N], F32, tag="hT")
        hT_bf = sbig.tile([128, N], BF16, tag="hTb", bufs=2)
        for c in range(NCHUNK):
            xb = shd.tile([128, DIN], BF16, tag="xchb")
            nc.vector.tensor_copy(xb, x_chunks[c])
            pt = pmisc.tile([DIN, 128], BF16, tag="pbig")
            nc.tensor.transpose(pt, xb, ident_bf)
            nc.scalar.activation(xT[:, bass.ts(c, 128)], pt, AF.Copy)
            nc.tensor.matmul(ph[:, bass.ts(c, 128)], Wp_sb, xT[:, bass.ts(c, 128)],
                             start=True, stop=True)
            nc.scalar.activation(hT[:, bass.ts(c, 128)], ph[:, bass.ts(c, 128)],
                                 AF.Relu, bias=bp_col)
            nc.vector.tensor_copy(hT_bf[:, bass.ts(c, 128)], hT[:, bass.ts(c, 128)])

        # ---------------- adj -> maskT (bf16, transposed) ----------------
        adj_f = []
        for r in range(NCHUNK):
            ai = shd.tile([128, N], mybir.dt.int32, tag="adji")
            nc.scalar.dma_start(ai, adj_d[bass.ts(r, 128), :])
            af = smask.tile([128, N], BF16, tag="adjf")
            nc.vector.tensor_copy(af, ai)
            adj_f.append(af)
        maskT_all = const.tile([128, NCHUNK, N], BF16)
        maskT = [maskT_all[:, c, :] for c in range(NCHUNK)]
        for r in range(NCHUNK):
            for c in range(NCHUNK):
                pm = pmisc.tile([128, 128], BF16, tag="pbig")
                nc.tensor.transpose(pm, adj_f[r][:, bass.ts(c, 128)], ident_bf)
                nc.scalar.activation(maskT[c][:, bass.ts(r, 128)], pm, AF.Copy)

        # ------------- attention body (shared by heads & out-att) -----------
        def attention(e1b_sb, e2i_cols, e2_cols, projNp, hid, out_f32=False):
            """e1b_sb: [128,N] bf16 bcast of E1b row.  e2i_cols/e2_cols: 4
            [128,1] col APs (E2inv f32 / E2 bf16).  projNp: [128,NCHUNK,128]
            bf16 AV lhsT already scaled by E2[m].  Returns outT = pou/den."""
            t_m = sexp.tile([128, NCHUNK, N], BF16, tag="t_m", bufs=3)
            s_t = sexp.tile([128, NCHUNK, N], BF16, tag="s_t", bufs=6)
            for c in range(NCHUNK):
                nc.vector.tensor_scalar_max(t_m[:, c, :], e1b_sb, e2i_cols[c])
            nc.vector.tensor_tensor(s_t, t_m, maskT_all, OP.mult)
            den_ps = prow.tile([1, N], F32, tag="prow")
            for c in range(NCHUNK):
                nc.tensor.matmul(den_ps, e2_cols[c], s_t[:, c, :],
                                 start=(c == 0), stop=(c == NCHUNK - 1))
            pou_ps = pou.tile([128, N], F32, tag="oU")
            for c in range(NCHUNK):
                nc.tensor.matmul(pou_ps, projNp[:, c, :], s_t[:, c, :],
                                 start=(c == 0), stop=(c == NCHUNK - 1))
            rrow = srow.tile([1, N], F32, tag="rrowf")
            nc.vector.reciprocal_approx_fast(rrow, den_ps)
            rrow_bf = srow.tile([1, N], BF16, tag="rrowb")
            nc.vector.tensor_copy(rrow_bf, rrow)
            rep_ps = prow.tile([128, N], F32, tag="prow")
            nc.tensor.matmul(rep_ps, ones_row_bf[:, 0:128], rrow_bf,
                             start=True, stop=True)
            rep = sbcast.tile([128, N], BF16, tag="rep", bufs=6)
            nc.scalar.activation(rep, rep_ps, AF.Copy)
            pou_bf = shd.tile([128, N], BF16, tag="poubf")
            nc.scalar.activation(pou_bf, pou_ps, AF.Copy)
            outT = sbig.tile([128, N], F32 if out_f32 else BF16, tag="outT",
                             bufs=4)
            nc.vector.tensor_tensor(outT, pou_bf, rep, OP.mult)
            return outT

        # ---------------- layers ----------------
        for l in range(L):
            residT = hT
            # --- rows for all heads: s12[2h] = s1_h, s12[2h+1] = s2_h
            s12_ps = prow.tile([2 * H, N], F32, tag="prow")
            nc.tensor.matmul(s12_ps, Wt_bf[l], hT_bf, start=True, stop=True)
            Eblk = sx.tile([16, N], BF16, tag="Eblk")   # exp(+beta*s): rows 2h = E1b
            nc.scalar.activation(Eblk, s12_ps, AF.Exp, scale=BETA)
            # E1b broadcasts: one DRAM bounce write of all rows, then one
            # stride-0 broadcast read per head, spread across DMA queues
            dmaq = [nc.sync, nc.scalar, nc.gpsimd]
            nc.sync.dma_start(ebl_d[l], Eblk)
            e1b = []
            for h in range(H):
                row = ebl_d[l][2 * h, :]
                src_bc = bass.AP(tensor=row.tensor, offset=row.offset,
                                 ap=[[0, 128], [1, N]])
                eb = sbcast.tile([128, N], BF16, tag="e1b")
                dmaq[h % 3].dma_start(eb, src_bc)
                e1b.append(eb)
            # --- s2 columns directly via tiny matmuls (no transposes)
            Wt2 = Wt_bf[l].rearrange("i (h t) -> i t h", t=2)[:, 1, :]
            cps = prow.tile([128, NCHUNK, 8], F32, tag="prow")
            for c in range(NCHUNK):
                nc.tensor.matmul(cps[:, c, :], hT_bf[:, bass.ts(c, 128)], Wt2,
                                 start=True, stop=True)
            C_e2i = sx.tile([128, NCHUNK, 8], F32, tag="Ce2i")
            nc.scalar.activation(C_e2i, cps, AF.Exp, scale=-BETA)
            C_e2b = sx.tile([128, NCHUNK, 8], BF16, tag="Ce2b")
            nc.scalar.activation(C_e2b, cps, AF.Exp, scale=1.0)
            C_e2f = sx.tile([128, NCHUNK, 8], F32, tag="Ce2f")
            nc.scalar.activation(C_e2f, cps, AF.Exp, scale=1.0)

            def e2i_col(h, c):
                return C_e2i[:, c, h:h + 1]

            def e2_col(h, c):
                return C_e2f[:, c, h:h + 1]

            def e2_col_bf(h, c):
                return C_e2b[:, c, h:h + 1]

            # --- projN: batched over heads (2 x 512-free MMs per chunk),
            # evacuated per head with E2[m] scale fused
            projNp = [sproj.tile([128, NCHUNK, 128], BF16, tag="projNp",
                                 name=f"pp{l}_{h}") for h in range(H)]
            WhV = Wh_ball[l].rearrange("i h f -> i (h f)")
            for c in range(NCHUNK):
                for g in range(2):
                    pN = pmisc.tile([128, N], F32, tag="pbig")
                    nc.tensor.matmul(pN, hT_bf[:, bass.ts(c, 128)],
                                     WhV[:, bass.ts(g, 512)], start=True, stop=True)
                    for j in range(4):
                        h = g * 4 + j
                        nc.scalar.activation(projNp[h][:, c, :],
                                             pN[:, bass.ts(j, 128)],
                                             AF.Identity, scale=e2_col(h, c))
            # --- attention per head + ELU
            multiT = []
            for h in range(H):
                outT = attention(
                    e1b[h],
                    [e2i_col(h, c) for c in range(NCHUNK)],
                    [e2_col_bf(h, c) for c in range(NCHUNK)],
                    projNp[h], l * 9 + h)
                ex = shd.tile([128, N], BF16, tag="elu_ex")
                nc.scalar.activation(ex, outT, AF.Exp)
                eng = nc.gpsimd if GP_ELU else nc.vector
                eng.tensor_scalar(ex, ex, 1.0, -1.0, OP.min, OP.add)
                mh = smulti.tile([128, N], BF16, tag="multi")
                eng2 = nc.gpsimd if GP_EMAX else nc.vector
                eng2.tensor_tensor(mh, outT, ex, OP.max)
                multiT.append(mh)

            # --- W_out projection
            ph2 = pou.tile([128, N], F32, tag="oU")
            for h in range(H):
                nc.tensor.matmul(ph2, Wo_bf[l][:, h, :], multiT[h],
                                 start=(h == 0), stop=(h == H - 1))
            h2_bf = sbig.tile([128, N], BF16, tag="h2b", bufs=2)
            nc.scalar.activation(h2_bf, ph2, AF.Copy)

            # --- single out-attention
            s12o_ps = prow.tile([2, N], F32, tag="prow")
            nc.tensor.matmul(s12o_ps, ao_bf[l], h2_bf, start=True, stop=True)
            Xo_b = sx.tile([1, N], BF16, tag="Xo_b")    # E1b_o row
            nc.scalar.activation(Xo_b, s12o_ps[0:1, :], AF.Exp, scale=BETA)
            so_ps = prow.tile([128, NCHUNK, 2], F32, tag="prow")
            for c in range(NCHUNK):
                nc.tensor.matmul(so_ps[:, c, :], h2_bf[:, bass.ts(c, 128)],
                                 ao_bf[l], start=True, stop=True)
            Co_e2i = sx.tile([128, NCHUNK, 2], F32, tag="Coe2i")
            nc.scalar.activation(Co_e2i, so_ps, AF.Exp, scale=-BETA)
            Co_e2b = sx.tile([128, NCHUNK, 2], BF16, tag="Coe2b")
            nc.scalar.activation(Co_e2b, so_ps, AF.Exp, scale=1.0)
            Co_e2f = sx.tile([128, NCHUNK, 2], F32, tag="Coe2f")
            nc.scalar.activation(Co_e2f, so_ps, AF.Exp, scale=1.0)
            # E1b_o broadcast via PE rank-1 (low latency; PE idle here)
            ebo_ps = pmisc.tile([128, N], F32, tag="pbig")
            nc.tensor.matmul(ebo_ps, ones_row_bf[:, 0:128], Xo_b, start=True, stop=True)
            e1bo = sbcast.tile([128, N], BF16, tag="e1b")
            nc.scalar.activation(e1bo, ebo_ps, AF.Copy)
            # h2N directly from multiT via accumulating matmuls (PE is idle
            # in the tail; starts as soon as multiT lands, no transpose chain)
            h2n_ps = pmisc.tile([128, N], F32, tag="pbig")
            for c in range(NCHUNK):
                for h in range(H):
                    nc.tensor.matmul(h2n_ps[:, bass.ts(c, 128)],
                                     multiT[h][:, bass.ts(c, 128)],
                                     Wo_bf[l][:, h, :],
                                     start=(h == 0), stop=(h == H - 1))
            h2Np = sproj.tile([128, NCHUNK, 128], BF16, tag="projNp")
            for c in range(NCHUNK):
                nc.scalar.activation(h2Np[:, c, :], h2n_ps[:, bass.ts(c, 128)],
                                     AF.Identity, scale=Co_e2f[:, c, 1:2])
            outsT = attention(
                e1bo,
                [Co_e2i[:, c, 1:2] for c in range(NCHUNK)],
                [Co_e2b[:, c, 1:2] for c in range(NCHUNK)],
                h2Np, l * 9 + 8, out_f32=True)

            # ---- residual + LN over partition dim ----
            xs = sbig.tile([128, N], F32, tag="xs", bufs=2)
            nc.vector.tensor_tensor(xs, outsT, residT, OP.add)
            xs_bf = sbig.tile([128, N], BF16, tag="xsqb", bufs=2)
            nc.vector.tensor_copy(xs_bf, xs)
            xsq = sbig.tile([128, N], BF16, tag="xsq", bufs=2)
            nc.scalar.activation(xsq, xs, AF.Square)
            pmu = prow.tile([1, N], F32, tag="prow")
            nc.tensor.matmul(pmu, ones_col_bf, xs_bf, start=True, stop=True)
            psq = prow.tile([1, N], F32, tag="prow")
            nc.tensor.matmul(psq, ones_col_bf, xsq, start=True, stop=True)
            mu = srow.tile([1, N], F32, tag="rowL")
            nc.vector.tensor_scalar_mul(mu, pmu, 1.0 / F)
            msq = srow.tile([1, N], F32, tag="rowL")
            nc.vector.tensor_scalar_mul(msq, psq, 1.0 / F)
            mu2 = srow.tile([1, N], F32, tag="rowL")
            nc.vector.tensor_tensor(mu2, mu, mu, OP.mult)
            var = srow.tile([1, N], F32, tag="rowL")
            nc.vector.tensor_tensor(var, msq, mu2, OP.subtract)
            lnv = srow.tile([1, N], F32, tag="rowL")
            nc.scalar.activation(lnv, var, AF.Ln, bias=eps1)
            rstd = srow.tile([1, N], F32, tag="rowL")
            nc.scalar.activation(rstd, lnv, AF.Exp, scale=-0.5)
            r2 = srow.tile([1, N], BF16, tag="rowLb")
            nc.vector.scalar_tensor_tensor(r2, mu, -1.0, rstd, OP.mult, OP.mult)
            rstd_bf = srow.tile([1, N], BF16, tag="rowLb")
            nc.vector.tensor_copy(rstd_bf, rstd)
            paff = pmisc.tile([128, N], F32, tag="pbig")
            nc.tensor.matmul(paff, g_row_bf[l], r2, start=True, stop=False)
            nc.tensor.matmul(paff, b_row_bf[l], ones_row_bf, start=False, stop=True)
            prs = pmisc.tile([128, N], F32, tag="pbig")
            nc.tensor.matmul(prs, g_row_bf[l], rstd_bf,
                             start=True, stop=True)
            rep_grstd = sbig.tile([128, N], F32, tag="repo", bufs=2)
            nc.scalar.activation(rep_grstd, prs, AF.Copy)
            y = sbig.tile([128, N], F32, tag="y", bufs=2)
            nc.vector.tensor_tensor(y, xs, rep_grstd, OP.mult)
            hT_new = sbig.tile([128, N], F32, tag="hT")
            nc.vector.tensor_tensor(hT_new, y, paff, OP.add)
            if l < L - 1:
                nc.vector.tensor_scalar_max(hT_new, hT_new, 0.0)
            hT = hT_new
            if l < L - 1:
                hT_bf = sbig.tile([128, N], BF16, tag="hTb", bufs=2)
                nc.vector.tensor_copy(hT_bf, hT)

        # ---------------- output: transpose back ----------------
        for c in range(NCHUNK):
            po = pmisc.tile([128, 128], F32, tag="pbig")
            nc.tensor.transpose(po, hT[:, bass.ts(c, 128)], ident)
            osb = shd.tile([128, 128], F32, tag="osb")
            nc.scalar.activation(osb, po, AF.Copy)
            nc.sync.dma_start(out_d[bass.ts(c, 128), :], osb)

    nc.compile()
    return nc


def _get_nc():
    if "nc" not in _CACHE:
        _CACHE["nc"] = build_nc()
    return _CACHE["nc"]


def kernel(**inputs) -> np.ndarray:
    nc = _get_nc()
    shared = {k: np.ascontiguousarray(np.asarray(inputs[k], dtype=np.float32))
              for k in ("Wp", "bp", "W_heads", "a_heads", "W_out", "a_out",
                        "ln_g", "ln_b")}
    x = np.asarray(inputs["x"], dtype=np.float32)
    adj = np.asarray(inputs["adj"], dtype=np.int32)
    in_maps = [dict(x=np.ascontiguousarray(x[b]),
                    adj=np.ascontiguousarray(adj[b]), **shared)
               for b in range(B)]
    res = run_bass_kernel_spmd(nc, in_maps, core_ids=list(range(B)))
    return np.stack([res.results[b]["out"] for b in range(B)])


if __name__ == "__main__":
    rng = np.random.default_rng(0)
    inputs = dict(
        x=rng.normal(size=(B, N, DIN)).astype(np.float32),
        adj=rng.integers(0, 2, size=(B, N, N)).astype(np.int32),
        Wp=(rng.normal(size=(DIN, F)) * 0.12).astype(np.float32),
        bp=np.zeros(F, dtype=np.float32),
        W_heads=(rng.normal(size=(L, H, F, F)) * 0.08).astype(np.float32),
        a_heads=(rng.normal(size=(L, H, 2 * F)) * 0.08).astype(np.float32),
        W_out=(rng.normal(size=(L, H * F, F)) * 0.03).astype(np.float32),
        a_out=(rng.normal(size=(L, 2 * F)) * 0.08).astype(np.float32),
        ln_g=np.ones((L, F), dtype=np.float32),
        ln_b=np.zeros((L, F), dtype=np.float32),
    )
    out = kernel(**inputs)
    print("out", out.shape, out.dtype, np.abs(out).max())



# revision 10
# speedup vs baseline: 1.0663x; 1.0663x over previous
"""GAT spatio-temporal model Trainium2 kernel (v6).

Sharding: data-parallel over batch B=8 -> 8 NeuronCores (1 graph each).

Attention factorization (exact): with E = exp(s), Ea = exp(alpha*s),
exp(lrelu(s1[n]+s2[m])) = max(E1[n]E2[m], E1a[n]E2a[m]).  Dividing by
E1a[n] (constant along the softmax axis, cancels):
    p[m,n] = max(E2[m]*E1b[n], E2a[m]) * mask[n,m],  E1b = exp(beta*s1)
so E2 is folded INTO the score tensor (v6):
 - t = tensor_scalar(e1b_bcast, *E2[m], max E2a[m]) -- 2x DVE mode
 - s_t = t * maskT (one batched [128,4N] tensor_tensor per head)
 - num = sum_m projN_plain[m,F] s_t[m,n]  (plain batched PSUM evacs)
 - den = sum_m s_t[m,n] via zero-padded ones lhsT, 4 heads -> one [4,N]
   PSUM tile -> ONE reciprocal + cast per group.
 - 1/den and E1b_o broadcasts via PE rank-1 + ACT evac (low latency).
 - h2N via PE transposes of h2_bf (not 32 small matmuls).
 - LN: stats for the 4 n-chunks land on 4 PSUM partitions (zero-padded
   1/F lhsT) so row ops run 4x fewer elements; affine+ReLU fused into
   one ACT (scale=g, bias=b per partition).

Shapes (hardcoded): B=8, N=512, Din=64, H=8, F=128, L=2.
"""
import os
import numpy as np
from contextlib import ExitStack

import concourse.bass as bass
import concourse.tile as tile
from concourse import bacc, mybir
from concourse.bass_utils import run_bass_kernel_spmd
from concourse.masks import make_identity

F32 = mybir.dt.float32
BF16 = mybir.dt.bfloat16
AF = mybir.ActivationFunctionType
OP = mybir.AluOpType

B, N, DIN, H, F, L = 8, 512, 64, 8, 128, 2
NCHUNK = N // 128  # 4
NG = 2             # den groups per layer (4 heads each)
GH = H // NG       # heads per group
ALPHA = 0.2
BETA = 1.0 - ALPHA
LN_EPS = 1e-5

_CACHE = {}


def build_nc():
    nc = bacc.Bacc("TRN2", target_bir_lowering=False, debug=False)

    x_d = nc.dram_tensor("x", [N, DIN], F32, kind="ExternalInput").ap()
    adj_d = nc.dram_tensor("adj", [N, N], mybir.dt.int32, kind="ExternalInput").ap()
    Wp_d = nc.dram_tensor("Wp", [DIN, F], F32, kind="ExternalInput").ap()
    bp_d = nc.dram_tensor("bp", [F], F32, kind="ExternalInput").ap()
    Wh_d = nc.dram_tensor("W_heads", [L, H, F, F], F32, kind="ExternalInput").ap()
    ah_d = nc.dram_tensor("a_heads", [L, H, 2 * F], F32, kind="ExternalInput").ap()
    Wo_d = nc.dram_tensor("W_out", [L, H * F, F], F32, kind="ExternalInput").ap()
    ao_d = nc.dram_tensor("a_out", [L, 2 * F], F32, kind="ExternalInput").ap()
    g_d = nc.dram_tensor("ln_g", [L, F], F32, kind="ExternalInput").ap()
    b_d = nc.dram_tensor("ln_b", [L, F], F32, kind="ExternalInput").ap()
    out_d = nc.dram_tensor("out", [N, F], F32, kind="ExternalOutput").ap()
    # DRAM bounce buffers for the E1b row broadcasts
    ebl_d = [nc.dram_tensor(f"eblk{l}", [16, N], BF16, kind="ExternalOutput").ap()
             for l in range(L)]

    with tile.TileContext(nc) as tc, ExitStack() as ctx:
        const = ctx.enter_context(tc.tile_pool(name="const", bufs=1))
        sx = ctx.enter_context(tc.tile_pool(name="sx", bufs=2))
        sproj = ctx.enter_context(tc.tile_pool(name="sproj", bufs=2))
        sbcast = ctx.enter_context(tc.tile_pool(name="sbcast", bufs=9))
        sexp = ctx.enter_context(tc.tile_pool(name="sexp", bufs=7))
        smulti = ctx.enter_context(tc.tile_pool(name="smulti", bufs=9))
        sbig = ctx.enter_context(tc.tile_pool(name="sbig", bufs=3))
        srow = ctx.enter_context(tc.tile_pool(name="srow", bufs=2))
        shd = ctx.enter_context(tc.tile_pool(name="shd", bufs=4))
        smask = ctx.enter_context(tc.tile_pool(name="smask", bufs=4))
        pou = ctx.enter_context(tc.tile_pool(name="pou", bufs=3, space="PSUM"))
        pmisc = ctx.enter_context(tc.tile_pool(name="pmisc", bufs=2, space="PSUM"))
        prow = ctx.enter_context(tc.tile_pool(name="prow", bufs=2, space="PSUM"))

        # ---------------- input DMAs first, spread across all hw queues ----
        x_chunks = []
        for c in range(NCHUNK):
            xc = shd.tile([128, DIN], F32, tag="xchunk")
            nc.sync.dma_start(xc, x_d[bass.ts(c, 128), :])
            x_chunks.append(xc)
        Wh_ball = [const.tile([F, H, F], BF16, name=f"WhB{l}") for l in range(L)]
        Wh0_f = const.tile([F, H, F], F32)
        nc.scalar.dma_start(Wh0_f, Wh_d[0].rearrange("h i o -> i h o"))
        adj_qs = [nc.sync, nc.scalar, nc.sync, nc.scalar]
        adj_raw = []
        for r in range(NCHUNK):
            ai = shd.tile([128, N], mybir.dt.int32, tag="adji", bufs=4)
            adj_qs[r].dma_start(ai, adj_d[bass.ts(r, 128), :])
            adj_raw.append(ai)
        Wp_sb = const.tile([DIN, F], BF16)
        nc.gpsimd.dma_start(Wp_sb, Wp_d)
        bp_col = const.tile([F, 1], F32)
        nc.sync.dma_start(bp_col, bp_d.rearrange("(f one) -> f one", one=1))
        ah_ball = const.tile([F, L * H, 2], BF16)
        nc.gpsimd.dma_start(ah_ball, ah_d.rearrange("l h (t f) -> f (l h) t", t=2))
        ah_bf = [[ah_ball[:, l * H + h, :] for h in range(H)] for l in range(L)]
        ao_ball = const.tile([F, L, 2], BF16)
        nc.gpsimd.dma_start(ao_ball, ao_d.rearrange("l (t f) -> f l t", t=2))
        ao_bf = [ao_ball[:, l, :] for l in range(L)]
        gb_all = const.tile([F, 2 * L], F32)
        nc.gpsimd.dma_start(gb_all[:, 0:L], g_d.rearrange("l f -> f l"))
        nc.gpsimd.dma_start(gb_all[:, L:2 * L], b_d.rearrange("l f -> f l"))
        g_col = [gb_all[:, l:l + 1] for l in range(L)]
        b_col = [gb_all[:, L + l:L + l + 1] for l in range(L)]
        # late-need weights on the software queue
        nc.gpsimd.dma_start(Wh_ball[1], Wh_d[1].rearrange("h i o -> i h o"))
        Wo_ball = [const.tile([128, H, F], BF16, name=f"WoB{l}") for l in range(L)]
        for l in range(L):
            nc.gpsimd.dma_start(Wo_ball[l], Wo_d[l].rearrange("(c p) f -> p c f", p=128))
        Wo_bf = Wo_ball

        # ---------------- constants ----------------
        ones_row_bf = const.tile([1, N], BF16)
        nc.vector.memset(ones_row_bf, 1.0)
        ones_col_bf = const.tile([128, 1], BF16)
        nc.vector.memset(ones_col_bf, 1.0)
        ident = const.tile([128, 128], F32)
        make_identity(nc, ident)
        ident_bf = const.tile([128, 128], BF16)
        nc.vector.tensor_copy(ident_bf, ident)
        eps_col = const.tile([128, 1], F32)
        nc.vector.memset(eps_col, LN_EPS)
        # onespad[:, j, k] = 1 iff k == j  (den-group lhsT)
        onespad = const.tile([128, GH, GH], BF16)
        nc.vector.memset(onespad, 0.0)
        for j in range(GH):
            nc.vector.memset(onespad[:, j, j:j + 1], 1.0)
        # invFpad[:, c, k] = 1/F iff k == c  (LN 4-partition stats lhsT)
        invFpad = const.tile([128, NCHUNK, NCHUNK], BF16)
        nc.vector.memset(invFpad, 0.0)
        for c in range(NCHUNK):
            nc.vector.memset(invFpad[:, c, c:c + 1], 1.0 / F)
        # sel4[k, j, :] = 1 iff k == j: row-selector lhsT for rank-1
        # broadcasts out of [4, N] tiles (rhs base partition must be 0)
        sel4 = const.tile([4, NCHUNK, 128], BF16)
        nc.gpsimd.memset(sel4, 0.0)
        nc.gpsimd.affine_select(
            out=sel4, in_=sel4, compare_op=OP.not_equal, fill=1.0,
            base=0, pattern=[[-1, NCHUNK], [0, 128]], channel_multiplier=1)

        nc.vector.tensor_copy(Wh_ball[0], Wh0_f)
        Wh_bf = [[Wh_ball[l][:, h, :] for h in range(H)] for l in range(L)]

        # ------------- WhT (transposed head weights) + Wtilde = W @ a -------
        WhT_ball = [const.tile([F, H, F], BF16, name=f"WhT{l}") for l in range(L)]
        for l in range(L):
            for h in range(H):
                pt = pou.tile([128, 128], BF16, tag="oU")
                nc.tensor.transpose(pt, Wh_bf[l][h], ident_bf)
                if h % 2 == 0:
                    nc.scalar.activation(WhT_ball[l][:, h, :], pt, AF.Copy)
                else:
                    nc.vector.tensor_copy(WhT_ball[l][:, h, :], pt)
        Wt_bf = [const.tile([F, 2 * H], BF16, name=f"Wt{l}") for l in range(L)]
        for l in range(L):
            pw = prow.tile([128, 2 * H], F32, tag="prow")
            for h in range(H):
                nc.tensor.matmul(pw[:, 2 * h:2 * h + 2], WhT_ball[l][:, h, :],
                                 ah_bf[l][h], start=True, stop=True)
            nc.scalar.activation(Wt_bf[l], pw, AF.Copy)

        # ---------------- x -> xT, input projection ----------------
        xT = const.tile([DIN, N], BF16)
        ph = pmisc.tile([128, N], F32, tag="pbig")
        hT = sbig.tile([128, N], F32, tag="hT")
        hT_bf = sbig.tile([128, N], BF16, tag="hTb", bufs=2)
        for c in range(NCHUNK):
            xb = shd.tile([128, DIN], BF16, tag="xchb")
            nc.vector.tensor_copy(xb, x_chunks[c])
            pt = pmisc.tile([DIN, 128], BF16, tag="pbig")
            nc.tensor.transpose(pt, xb, ident_bf)
            nc.scalar.activation(xT[:, bass.ts(c, 128)], pt, AF.Copy)
            nc.tensor.matmul(ph[:, bass.ts(c, 128)], Wp_sb, xT[:, bass.ts(c, 128)],
                             start=True, stop=True)
            nc.scalar.activation(hT[:, bass.ts(c, 128)], ph[:, bass.ts(c, 128)],
                                 AF.Relu, bias=bp_col)
            nc.vector.tensor_copy(hT_bf[:, bass.ts(c, 128)], hT[:, bass.ts(c, 128)])

        # ---------------- adj -> maskT (bf16, transposed) ----------------
        adj_f = []
        for r in range(NCHUNK):
            af = smask.tile([128, N], BF16, tag="adjf")
            nc.vector.tensor_copy(af, adj_raw[r])
            adj_f.append(af)
        maskT_all = const.tile([128, NCHUNK, N], BF16)
        maskT = [maskT_all[:, c, :] for c in range(NCHUNK)]
        for r in range(NCHUNK):
            for c in range(NCHUNK):
                pm = pmisc.tile([128, 128], BF16, tag="pbig")
                nc.tensor.transpose(pm, adj_f[r][:, bass.ts(c, 128)], ident_bf)
                if (r + c) % 2 == 0:
                    nc.scalar.activation(maskT[c][:, bass.ts(r, 128)], pm, AF.Copy)
                else:
                    nc.vector.tensor_copy(maskT[c][:, bass.ts(r, 128)], pm)

        # ---------------- layers ----------------
        for l in range(L):
            residT = hT
            # --- rows for all heads: s12[2h] = s1_h, s12[2h+1] = s2_h
            s12_ps = prow.tile([2 * H, N], F32, tag="prow")
            nc.tensor.matmul(s12_ps, Wt_bf[l], hT_bf, start=True, stop=True)
            Eblk = sx.tile([16, N], BF16, tag="Eblk")   # exp(+beta*s): rows 2h = E1b
            nc.scalar.activation(Eblk, s12_ps, AF.Exp, scale=BETA)
            # E1b broadcasts: one DRAM bounce write of all rows, then one
            # stride-0 broadcast read per head, spread across DMA queues
            dmaq = [nc.sync, nc.scalar, nc.gpsimd]
            nc.sync.dma_start(ebl_d[l], Eblk)
            e1b = []
            for h in range(H):
                row = ebl_d[l][2 * h, :]
                src_bc = bass.AP(tensor=row.tensor, offset=row.offset,
                                 ap=[[0, 128], [1, N]])
                eb = sbcast.tile([128, N], BF16, tag="e1b", bufs=9)
                dmaq[h % 3].dma_start(eb, src_bc)
                e1b.append(eb)
            # --- s2 columns directly via tiny matmuls (no transposes)
            Wt2 = Wt_bf[l].rearrange("i (h t) -> i t h", t=2)[:, 1, :]
            cps = prow.tile([128, NCHUNK, 8], F32, tag="prow")
            for c in range(NCHUNK):
                nc.tensor.matmul(cps[:, c, :], hT_bf[:, bass.ts(c, 128)], Wt2,
                                 start=True, stop=True)
            C_e2f = sx.tile([128, NCHUNK, 8], F32, tag="Ce2f")
            nc.scalar.activation(C_e2f, cps, AF.Exp, scale=1.0)
            C_e2a = sx.tile([128, NCHUNK, 8], F32, tag="Ce2a")
            nc.scalar.activation(C_e2a, cps, AF.Exp, scale=ALPHA)

            def e2_col(h, c):
                return C_e2f[:, c, h:h + 1]

            def e2a_col(h, c):
                return C_e2a[:, c, h:h + 1]

            # --- projN: batched over heads (2 x 512-free MMs per chunk),
            # plain batched evacuation (E2 lives in s_t now)
            projAll = sproj.tile([128, NCHUNK, H * 128], BF16, tag="projAll",
                                 name=f"pa{l}", bufs=1)
            WhV = Wh_ball[l].rearrange("i h f -> i (h f)")
            for c in range(NCHUNK):
                for g in range(2):
                    pN = pmisc.tile([128, N], F32, tag="pbig")
                    nc.tensor.matmul(pN, hT_bf[:, bass.ts(c, 128)],
                                     WhV[:, bass.ts(g, 512)], start=True, stop=True)
                    dst = projAll[:, c, g * 512:(g + 1) * 512]
                    if (c + g) % 2 == 0:
                        nc.scalar.activation(dst, pN, AF.Copy)
                    else:
                        nc.vector.tensor_copy(dst, pN)

            def proj_ct(h, c):
                return projAll[:, c, h * 128:(h + 1) * 128]

            # --- attention per group of GH heads
            pous = [None] * H
            reps = [None] * H
            for g in range(NG):
                deng_ps = prow.tile([GH, N], F32, tag="deng", bufs=1)
                for j in range(GH):
                    h = g * GH + j
                    tten = sexp.tile([128, NCHUNK, N], BF16, tag="tten", bufs=3)
                    for c in range(NCHUNK):
                        nc.vector.tensor_scalar(tten[:, c, :], e1b[h],
                                                e2_col(h, c), e2a_col(h, c),
                                                OP.mult, OP.max)
                    s_t = sexp.tile([128, NCHUNK, N], BF16, tag="s_t")
                    nc.vector.tensor_tensor(s_t, tten, maskT_all, OP.mult)
                    for c in range(NCHUNK):
                        nc.tensor.matmul(deng_ps, onespad[:, j, :], s_t[:, c, :],
                                         start=(j == 0 and c == 0),
                                         stop=(j == GH - 1 and c == NCHUNK - 1))
                    pou_ps = pou.tile([128, N], F32, tag="oU")
                    for c in range(NCHUNK):
                        nc.tensor.matmul(pou_ps, proj_ct(h, c), s_t[:, c, :],
                                         start=(c == 0), stop=(c == NCHUNK - 1))
                    pob = smulti.tile([128, N], BF16, tag="pob", bufs=8)
                    if h % 2 == 0:
                        nc.scalar.activation(pob, pou_ps, AF.Copy)
                    else:
                        nc.vector.tensor_copy(pob, pou_ps)
                    pous[h] = pob
                rr4 = srow.tile([GH, N], F32, tag="rr4")
                nc.vector.reciprocal_approx_fast(rr4, deng_ps)
                rr4_bf = srow.tile([GH, N], BF16, tag="rr4b")
                nc.vector.tensor_copy(rr4_bf, rr4)
                for j in range(GH):
                    h = g * GH + j
                    rep_ps = pmisc.tile([128, N], F32, tag="pbig")
                    nc.tensor.matmul(rep_ps, sel4[:, j, :], rr4_bf,
                                     start=True, stop=True)
                    rp = sbcast.tile([128, N], BF16, tag="rep", bufs=6)
                    nc.scalar.activation(rp, rep_ps, AF.Copy)
                    reps[h] = rp

            # --- normalize + ELU per head
            multiT = []
            for h in range(H):
                outT = sbig.tile([128, N], BF16, tag="outT", bufs=3)
                nc.vector.tensor_tensor(outT, pous[h], reps[h], OP.mult)
                ex = shd.tile([128, N], BF16, tag="elu_ex")
                nc.scalar.activation(ex, outT, AF.Exp)
                nc.vector.tensor_scalar(ex, ex, 1.0, -1.0, OP.min, OP.add)
                mh = smulti.tile([128, N], BF16, tag="multi")
                nc.vector.tensor_tensor(mh, outT, ex, OP.max)
                multiT.append(mh)

            # --- W_out projection (h2 in F-layout)
            ph2 = pou.tile([128, N], F32, tag="oU")
            for h in range(H):
                nc.tensor.matmul(ph2, Wo_bf[l][:, h, :], multiT[h],
                                 start=(h == 0), stop=(h == H - 1))
            h2_bf = sbig.tile([128, N], BF16, tag="h2b", bufs=2)
            nc.scalar.activation(h2_bf, ph2, AF.Copy)

            # --- single out-attention
            s12o_ps = prow.tile([2, N], F32, tag="prow")
            nc.tensor.matmul(s12o_ps, ao_bf[l], h2_bf, start=True, stop=True)
            Xo_b = sx.tile([1, N], BF16, tag="Xo_b")    # E1b_o row
            nc.scalar.activation(Xo_b, s12o_ps[0:1, :], AF.Exp, scale=BETA)
            ebo_ps = pmisc.tile([128, N], F32, tag="pbig")
            nc.tensor.matmul(ebo_ps, ones_row_bf[:, 0:128], Xo_b,
                             start=True, stop=True)
            e1bo = sbcast.tile([128, N], BF16, tag="e1b", bufs=9)
            nc.scalar.activation(e1bo, ebo_ps, AF.Copy)
            so_ps = prow.tile([128, NCHUNK, 2], F32, tag="prow")
            for c in range(NCHUNK):
                nc.tensor.matmul(so_ps[:, c, :], h2_bf[:, bass.ts(c, 128)],
                                 ao_bf[l], start=True, stop=True)
            Co_e2f = sx.tile([128, NCHUNK, 2], F32, tag="Coe2f")
            nc.scalar.activation(Co_e2f, so_ps, AF.Exp, scale=1.0)
            Co_e2a = sx.tile([128, NCHUNK, 2], F32, tag="Coe2a")
            nc.scalar.activation(Co_e2a, so_ps, AF.Exp, scale=ALPHA)
            # h2N via PE transposes of h2_bf, plain evac (E2_o lives in s_to)
            h2Np = sproj.tile([128, NCHUNK, 128], BF16, tag="h2Np")
            for c in range(NCHUNK):
                pm = pmisc.tile([128, 128], BF16, tag="pbig")
                nc.tensor.transpose(pm, h2_bf[:, bass.ts(c, 128)], ident_bf)
                nc.scalar.activation(h2Np[:, c, :], pm, AF.Copy)
            # out-att scores
            tto = sexp.tile([128, NCHUNK, N], BF16, tag="tten", bufs=3)
            for c in range(NCHUNK):
                nc.vector.tensor_scalar(tto[:, c, :], e1bo,
                                        Co_e2f[:, c, 1:2], Co_e2a[:, c, 1:2],
                                        OP.mult, OP.max)
            s_to = sexp.tile([128, NCHUNK, N], BF16, tag="s_t")
            nc.vector.tensor_tensor(s_to, tto, maskT_all, OP.mult)
            deno_ps = prow.tile([1, N], F32, tag="prow")
            for c in range(NCHUNK):
                nc.tensor.matmul(deno_ps, ones_col_bf, s_to[:, c, :],
                                 start=(c == 0), stop=(c == NCHUNK - 1))
            pouo_ps = pou.tile([128, N], F32, tag="oU")
            for c in range(NCHUNK):
                nc.tensor.matmul(pouo_ps, h2Np[:, c, :], s_to[:, c, :],
                                 start=(c == 0), stop=(c == NCHUNK - 1))
            rro = srow.tile([1, N], F32, tag="rro")
            nc.vector.reciprocal_approx_fast(rro, deno_ps)
            rro_bf = srow.tile([1, N], BF16, tag="rrob")
            nc.vector.tensor_copy(rro_bf, rro)
            rpo_ps = pmisc.tile([128, N], F32, tag="pbig")
            nc.tensor.matmul(rpo_ps, ones_row_bf[:, 0:128], rro_bf,
                             start=True, stop=True)
            repo = sbcast.tile([128, N], BF16, tag="rep", bufs=6)
            nc.scalar.activation(repo, rpo_ps, AF.Copy)
            pobo = smulti.tile([128, N], BF16, tag="pob", bufs=8)
            nc.vector.tensor_copy(pobo, pouo_ps)
            outsT = sbig.tile([128, N], BF16, tag="outsT", bufs=2)
            nc.vector.tensor_tensor(outsT, pobo, repo, OP.mult)

            # ---- residual + LN over partition dim ----
            xs = sbig.tile([128, N], F32, tag="xs", bufs=2)
            nc.vector.tensor_tensor(xs, outsT, residT, OP.add)
            xs_bf = sbig.tile([128, N], BF16, tag="xsqb", bufs=2)
            nc.vector.tensor_copy(xs_bf, xs)
            xsq = sbig.tile([128, N], BF16, tag="xsq", bufs=2)
            nc.scalar.activation(xsq, xs, AF.Square)
            # 4-partition stats: row c of [4,128] = mean/meansq of chunk c
            pmu4 = prow.tile([NCHUNK, 128], F32, tag="prow")
            for c in range(NCHUNK):
                nc.tensor.matmul(pmu4, invFpad[:, c, :], xs_bf[:, bass.ts(c, 128)],
                                 start=(c == 0), stop=(c == NCHUNK - 1))
            psq4 = prow.tile([NCHUNK, 128], F32, tag="prow")
            for c in range(NCHUNK):
                nc.tensor.matmul(psq4, invFpad[:, c, :], xsq[:, bass.ts(c, 128)],
                                 start=(c == 0), stop=(c == NCHUNK - 1))
            mu2 = srow.tile([NCHUNK, 128], F32, tag="rowL", bufs=4)
            nc.scalar.activation(mu2, pmu4, AF.Square)
            nmu4_bf = srow.tile([NCHUNK, 128], BF16, tag="rowLb", bufs=2)
            nc.vector.tensor_scalar_mul(nmu4_bf, pmu4, -1.0)
            var4 = srow.tile([NCHUNK, 128], F32, tag="rowL", bufs=4)
            nc.vector.tensor_tensor(var4, psq4, mu2, OP.subtract)
            lnv4 = srow.tile([NCHUNK, 128], F32, tag="rowL", bufs=4)
            nc.scalar.activation(lnv4, var4, AF.Ln, bias=eps_col[0:NCHUNK, :])
            rstd4 = srow.tile([NCHUNK, 128], F32, tag="rowL", bufs=4)
            nc.scalar.activation(rstd4, lnv4, AF.Exp, scale=-0.5)
            rstd4_bf = srow.tile([NCHUNK, 128], BF16, tag="rowLb", bufs=2)
            nc.vector.tensor_copy(rstd4_bf, rstd4)
            rep_rstd = pmisc.tile([128, N], F32, tag="pbig")
            rep_nmu = pmisc.tile([128, N], F32, tag="pbig")
            for c in range(NCHUNK):
                nc.tensor.matmul(rep_rstd[:, bass.ts(c, 128)],
                                 sel4[:, c, :], rstd4_bf,
                                 start=True, stop=True)
                nc.tensor.matmul(rep_nmu[:, bass.ts(c, 128)],
                                 sel4[:, c, :], nmu4_bf,
                                 start=True, stop=True)
            u = sbig.tile([128, N], F32, tag="u", bufs=2)
            nc.vector.tensor_tensor(u, xs, rep_nmu, OP.add)
            t2 = sbig.tile([128, N], F32, tag="t2", bufs=2)
            nc.vector.tensor_tensor(t2, u, rep_rstd, OP.mult)
            hT_new = sbig.tile([128, N], F32, tag="hT")
            fn = AF.Relu if l < L - 1 else AF.Identity
            nc.scalar.activation(hT_new, t2, fn, scale=g_col[l], bias=b_col[l])
            hT = hT_new
            if l < L - 1:
                hT_bf = sbig.tile([128, N], BF16, tag="hTb", bufs=2)
                nc.vector.tensor_copy(hT_bf, hT)

        # ---------------- output: transpose back ----------------
        for c in range(NCHUNK):
            po = pmisc.tile([128, 128], F32, tag="pbig")
            nc.tensor.transpose(po, hT[:, bass.ts(c, 128)], ident)
            osb = shd.tile([128, 128], F32, tag="osb")
            nc.scalar.activation(osb, po, AF.Copy)
            nc.sync.dma_start(out_d[bass.ts(c, 128), :], osb)

    nc.compile()
    return nc


def _get_nc():
    if "nc" not in _CACHE:
        _CACHE["nc"] = build_nc()
    return _CACHE["nc"]


def kernel(**inputs) -> np.ndarray:
    nc = _get_nc()
    shared = {k: np.ascontiguousarray(np.asarray(inputs[k], dtype=np.float32))
              for k in ("Wp", "bp", "W_heads", "a_heads", "W_out", "a_out",
                        "ln_g", "ln_b")}
    x = np.asarray(inputs["x"], dtype=np.float32)
    adj = np.asarray(inputs["adj"], dtype=np.int32)
    in_maps = [dict(x=np.ascontiguousarray(x[b]),
                    adj=np.ascontiguousarray(adj[b]), **shared)
               for b in range(B)]
    res = run_bass_kernel_spmd(nc, in_maps, core_ids=list(range(B)))
    return np.stack([res.results[b]["out"] for b in range(B)])


if __name__ == "__main__":
    rng = np.random.default_rng(0)
    inputs = dict(
        x=rng.normal(size=(B, N, DIN)).astype(np.float32),
        adj=rng.integers(0, 2, size=(B, N, N)).astype(np.int32),
        Wp=(rng.normal(size=(DIN, F)) * 0.12).astype(np.float32),
        bp=np.zeros(F, dtype=np.float32),
        W_heads=(rng.normal(size=(L, H, F, F)) * 0.08).astype(np.float32),
        a_heads=(rng.normal(size=(L, H, 2 * F)) * 0.08).astype(np.float32),
        W_out=(rng.normal(size=(L, H * F, F)) * 0.03).astype(np.float32),
        a_out=(rng.normal(size=(L, 2 * F)) * 0.08).astype(np.float32),
        ln_g=np.ones((L, F), dtype=np.float32),
        ln_b=np.zeros((L, F), dtype=np.float32),
    )
    out = kernel(**inputs)
    print("out", out.shape, out.dtype, np.abs(out).max())


# revision 13
# speedup vs baseline: 1.1812x; 1.1077x over previous
"""GAT spatio-temporal model Trainium2 kernel (v6).

Sharding: data-parallel over batch B=8 -> 8 NeuronCores (1 graph each).

Attention factorization (exact): with E = exp(s), Ea = exp(alpha*s),
exp(lrelu(s1[n]+s2[m])) = max(E1[n]E2[m], E1a[n]E2a[m]).  Dividing by
E1a[n] (constant along the softmax axis, cancels):
    p[m,n] = max(E2[m]*E1b[n], E2a[m]) * mask[n,m],  E1b = exp(beta*s1)
so E2 is folded INTO the score tensor (v6):
 - t = tensor_scalar(e1b_bcast, *E2[m], max E2a[m]) -- 2x DVE mode
 - s_t = t * maskT (one batched [128,4N] tensor_tensor per head)
 - num = sum_m projN_plain[m,F] s_t[m,n]  (plain batched PSUM evacs)
 - den = sum_m s_t[m,n] via zero-padded ones lhsT, 4 heads -> one [4,N]
   PSUM tile -> ONE reciprocal + cast per group.
 - 1/den and E1b_o broadcasts via PE rank-1 + ACT evac (low latency).
 - h2N via PE transposes of h2_bf (not 32 small matmuls).
 - LN: stats for the 4 n-chunks land on 4 PSUM partitions (zero-padded
   1/F lhsT) so row ops run 4x fewer elements; affine+ReLU fused into
   one ACT (scale=g, bias=b per partition).

Shapes (hardcoded): B=8, N=512, Din=64, H=8, F=128, L=2.
"""
import os
import numpy as np
from contextlib import ExitStack

import concourse.bass as bass
import concourse.tile as tile
from concourse import bacc, mybir
from concourse.bass_utils import run_bass_kernel_spmd
from concourse.masks import make_identity

F32 = mybir.dt.float32
BF16 = mybir.dt.bfloat16
AF = mybir.ActivationFunctionType
OP = mybir.AluOpType

B, N, DIN, H, F, L = 8, 512, 64, 8, 128, 2
NCHUNK = N // 128  # 4
NG = 2             # den groups per layer (4 heads each)
GH = H // NG       # heads per group
ALPHA = 0.2
BETA = 1.0 - ALPHA
LN_EPS = 1e-5

_CACHE = {}


def build_nc():
    nc = bacc.Bacc("TRN2", target_bir_lowering=False, debug=False)

    x_d = nc.dram_tensor("x", [N, DIN], F32, kind="ExternalInput").ap()
    adj_d = nc.dram_tensor("adj", [N, N], mybir.dt.int32, kind="ExternalInput").ap()
    Wp_d = nc.dram_tensor("Wp", [DIN, F], F32, kind="ExternalInput").ap()
    bp_d = nc.dram_tensor("bp", [F], F32, kind="ExternalInput").ap()
    Wh_d = nc.dram_tensor("W_heads", [L, H, F, F], F32, kind="ExternalInput").ap()
    ah_d = nc.dram_tensor("a_heads", [L, H, 2 * F], F32, kind="ExternalInput").ap()
    Wo_d = nc.dram_tensor("W_out", [L, H * F, F], F32, kind="ExternalInput").ap()
    ao_d = nc.dram_tensor("a_out", [L, 2 * F], F32, kind="ExternalInput").ap()
    g_d = nc.dram_tensor("ln_g", [L, F], F32, kind="ExternalInput").ap()
    b_d = nc.dram_tensor("ln_b", [L, F], F32, kind="ExternalInput").ap()
    out_d = nc.dram_tensor("out", [N, F], F32, kind="ExternalOutput").ap()
    # DRAM bounce buffers for the E1b row broadcasts
    ebl_d = [nc.dram_tensor(f"eblk{l}", [16, N], BF16, kind="ExternalOutput").ap()
             for l in range(L)]

    with tile.TileContext(nc) as tc, ExitStack() as ctx:
        const = ctx.enter_context(tc.tile_pool(name="const", bufs=1))
        sx = ctx.enter_context(tc.tile_pool(name="sx", bufs=2))
        sproj = ctx.enter_context(tc.tile_pool(name="sproj", bufs=2))
        sbcast = ctx.enter_context(tc.tile_pool(name="sbcast", bufs=9))
        sexp = ctx.enter_context(tc.tile_pool(name="sexp", bufs=7))
        smulti = ctx.enter_context(tc.tile_pool(name="smulti", bufs=9))
        sbig = ctx.enter_context(tc.tile_pool(name="sbig", bufs=3))
        srow = ctx.enter_context(tc.tile_pool(name="srow", bufs=2))
        shd = ctx.enter_context(tc.tile_pool(name="shd", bufs=4))
        smask = ctx.enter_context(tc.tile_pool(name="smask", bufs=4))
        pou = ctx.enter_context(tc.tile_pool(name="pou", bufs=2, space="PSUM"))
        pwarm = ctx.enter_context(tc.tile_pool(name="pwarm", bufs=1, space="PSUM"))
        pmisc = ctx.enter_context(tc.tile_pool(name="pmisc", bufs=2, space="PSUM"))
        prow = ctx.enter_context(tc.tile_pool(name="prow", bufs=2, space="PSUM"))

        # ---------------- input DMAs first, spread across all hw queues ----
        x_chunks = []
        for c in range(NCHUNK):
            xc = shd.tile([128, DIN], F32, tag="xchunk")
            nc.sync.dma_start(xc, x_d[bass.ts(c, 128), :])
            x_chunks.append(xc)
        Wh_ball = [const.tile([F, H, F], BF16, name=f"WhB{l}") for l in range(L)]
        Wh0_f = const.tile([F, H, F], F32)
        nc.scalar.dma_start(Wh0_f, Wh_d[0].rearrange("h i o -> i h o"))
        adj_qs = [nc.sync, nc.scalar, nc.sync, nc.scalar]
        adj_raw = []
        for r in range(NCHUNK):
            ai = shd.tile([128, N], mybir.dt.int32, tag="adji", bufs=4)
            adj_qs[r].dma_start(ai, adj_d[bass.ts(r, 128), :])
            adj_raw.append(ai)
        bp_col = const.tile([F, 1], F32)
        nc.sync.dma_start(bp_col, bp_d.rearrange("(f one) -> f one", one=1))

        # ---------------- constants (before gpsimd queue work) ----------------
        ones_row_bf = const.tile([1, N], BF16)
        nc.vector.memset(ones_row_bf, 1.0)
        ones_col_bf = const.tile([128, 1], BF16)
        nc.vector.memset(ones_col_bf, 1.0)
        ident = const.tile([128, 128], F32)
        make_identity(nc, ident)
        ident_bf = const.tile([128, 128], BF16)
        nc.vector.tensor_copy(ident_bf, ident)
        eps_col = const.tile([128, 1], F32)
        nc.vector.memset(eps_col, LN_EPS)
        # onespad[:, j, k] = 1 iff k == j  (den-group lhsT)
        onespad = const.tile([128, GH, GH], BF16)
        nc.vector.memset(onespad, 0.0)
        for j in range(GH):
            nc.vector.memset(onespad[:, j, j:j + 1], 1.0)
        # invFpad[:, c, k] = 1/F iff k == c  (LN 4-partition stats lhsT)
        invFpad = const.tile([128, NCHUNK, NCHUNK], BF16)
        nc.vector.memset(invFpad, 0.0)
        for c in range(NCHUNK):
            nc.vector.memset(invFpad[:, c, c:c + 1], 1.0 / F)
        # sel4[k, j, :] = 1 iff k == j: row-selector lhsT for rank-1
        # broadcasts out of [4, N] tiles (rhs base partition must be 0)
        sel4 = const.tile([4, NCHUNK, 128], BF16)
        nc.gpsimd.memset(sel4, 0.0)
        nc.gpsimd.affine_select(
            out=sel4, in_=sel4, compare_op=OP.not_equal, fill=1.0,
            base=0, pattern=[[-1, NCHUNK], [0, 128]], channel_multiplier=1)

        # gpsimd software-queue weight loads (after the const builds so the
        # identity/selector are ready early for the PE transposes)
        Wp_sb = const.tile([DIN, F], BF16)
        nc.gpsimd.dma_start(Wp_sb, Wp_d)
        ah_ball = const.tile([F, L * H, 2], BF16)
        nc.gpsimd.dma_start(ah_ball, ah_d.rearrange("l h (t f) -> f (l h) t", t=2))
        ah_bf = [[ah_ball[:, l * H + h, :] for h in range(H)] for l in range(L)]
        ao_ball = const.tile([F, L, 2], BF16)
        nc.gpsimd.dma_start(ao_ball, ao_d.rearrange("l (t f) -> f l t", t=2))
        ao_bf = [ao_ball[:, l, :] for l in range(L)]
        gb_all = const.tile([F, 2 * L], F32)
        nc.gpsimd.dma_start(gb_all[:, 0:L], g_d.rearrange("l f -> f l"))
        nc.gpsimd.dma_start(gb_all[:, L:2 * L], b_d.rearrange("l f -> f l"))
        g_col = [gb_all[:, l:l + 1] for l in range(L)]
        b_col = [gb_all[:, L + l:L + l + 1] for l in range(L)]
        nc.gpsimd.dma_start(Wh_ball[1], Wh_d[1].rearrange("h i o -> i h o"))
        Wo_ball = [const.tile([128, H, F], BF16, name=f"WoB{l}") for l in range(L)]
        for l in range(L):
            nc.gpsimd.dma_start(Wo_ball[l], Wo_d[l].rearrange("(c p) f -> p c f", p=128))
        Wo_bf = Wo_ball

        nc.vector.tensor_copy(Wh_ball[0], Wh0_f)
        Wh_bf = [[Wh_ball[l][:, h, :] for h in range(H)] for l in range(L)]

        # ------------- WhT (transposed head weights) + Wtilde = W @ a -------
        WhT_ball = [const.tile([F, H, F], BF16, name=f"WhT{l}") for l in range(L)]
        for l in range(L):
            for h in range(H):
                pt = pou.tile([128, 128], BF16, tag="oU")
                nc.tensor.transpose(pt, Wh_bf[l][h], ident_bf)
                if h % 2 == 0:
                    nc.scalar.activation(WhT_ball[l][:, h, :], pt, AF.Copy)
                else:
                    nc.vector.tensor_copy(WhT_ball[l][:, h, :], pt)
        Wt_bf = [const.tile([F, 2 * H], BF16, name=f"Wt{l}") for l in range(L)]
        for l in range(L):
            pw = prow.tile([128, 2 * H], F32, tag="prow")
            for h in range(H):
                nc.tensor.matmul(pw[:, 2 * h:2 * h + 2], WhT_ball[l][:, h, :],
                                 ah_bf[l][h], start=True, stop=True)
            nc.scalar.activation(Wt_bf[l], pw, AF.Copy)

        # ---------------- x -> xT, input projection ----------------
        xT = const.tile([DIN, N], BF16)
        ph = pmisc.tile([128, N], F32, tag="pbig")
        hT_bf = sbig.tile([128, N], BF16, tag="hTb", bufs=2)
        for c in range(NCHUNK):
            xb = shd.tile([128, DIN], BF16, tag="xchb")
            nc.vector.tensor_copy(xb, x_chunks[c])
            pt = pmisc.tile([DIN, 128], BF16, tag="pbig")
            nc.tensor.transpose(pt, xb, ident_bf)
            nc.scalar.activation(xT[:, bass.ts(c, 128)], pt, AF.Copy)
            nc.tensor.matmul(ph[:, bass.ts(c, 128)], Wp_sb, xT[:, bass.ts(c, 128)],
                             start=True, stop=True)
            nc.scalar.activation(hT_bf[:, bass.ts(c, 128)], ph[:, bass.ts(c, 128)],
                                 AF.Relu, bias=bp_col)
        hT = hT_bf

        # ---------------- adj -> maskT (bf16, transposed) ----------------
        adj_f = []
        for r in range(NCHUNK):
            af = smask.tile([128, N], BF16, tag="adjf")
            nc.vector.tensor_copy(af, adj_raw[r])
            adj_f.append(af)
        maskT_all = const.tile([128, NCHUNK, N], BF16)
        maskT = [maskT_all[:, c, :] for c in range(NCHUNK)]
        for r in range(NCHUNK):
            for c in range(NCHUNK):
                pm = pmisc.tile([128, 128], BF16, tag="pbig")
                nc.tensor.transpose(pm, adj_f[r][:, bass.ts(c, 128)], ident_bf)
                if (r + c) % 2 == 0:
                    nc.scalar.activation(maskT[c][:, bass.ts(r, 128)], pm, AF.Copy)
                else:
                    nc.vector.tensor_copy(maskT[c][:, bass.ts(r, 128)], pm)

        # HAM keep-warm: a tiny dependent matmul sprinkled into the serial
        # tail phases prevents the PE MID-window from seeing a fully idle
        # 3.4us window (which would re-throttle the clock to 1.2 GHz).
        def warm(dep_bf16_ap):
            wps = pwarm.tile([1, 1], F32, tag="warm")
            nc.tensor.matmul(wps, ones_col_bf[0:1, :], dep_bf16_ap[0:1, 0:1],
                             start=True, stop=True)

        # ---------------- layers ----------------
        for l in range(L):
            residT = hT
            # --- rows for all heads: s12[2h] = s1_h, s12[2h+1] = s2_h
            s12_ps = prow.tile([2 * H, N], F32, tag="prow")
            nc.tensor.matmul(s12_ps, Wt_bf[l], hT_bf, start=True, stop=True)
            Eblk = sx.tile([16, N], BF16, tag="Eblk")   # exp(+beta*s): rows 2h = E1b
            nc.scalar.activation(Eblk, s12_ps, AF.Exp, scale=BETA)
            # E1b broadcasts: one DRAM bounce write of all rows, then one
            # stride-0 broadcast read per head, spread across DMA queues
            dmaq = [nc.sync, nc.scalar, nc.gpsimd]
            nc.sync.dma_start(ebl_d[l], Eblk)
            e1b = []
            for h in range(H):
                row = ebl_d[l][2 * h, :]
                src_bc = bass.AP(tensor=row.tensor, offset=row.offset,
                                 ap=[[0, 128], [1, N]])
                eb = sbcast.tile([128, N], BF16, tag="e1b", bufs=9)
                dmaq[h % 3].dma_start(eb, src_bc)
                e1b.append(eb)
            # --- s2 columns directly via tiny matmuls (no transposes)
            Wt2 = Wt_bf[l].rearrange("i (h t) -> i t h", t=2)[:, 1, :]
            cps = prow.tile([128, NCHUNK, 8], F32, tag="prow")
            for c in range(NCHUNK):
                nc.tensor.matmul(cps[:, c, :], hT_bf[:, bass.ts(c, 128)], Wt2,
                                 start=True, stop=True)
            C_e2f = sx.tile([128, NCHUNK, 8], F32, tag="Ce2f")
            nc.scalar.activation(C_e2f, cps, AF.Exp, scale=1.0)
            C_e2a = sx.tile([128, NCHUNK, 8], F32, tag="Ce2a")
            nc.scalar.activation(C_e2a, cps, AF.Exp, scale=ALPHA)

            def e2_col(h, c):
                return C_e2f[:, c, h:h + 1]

            def e2a_col(h, c):
                return C_e2a[:, c, h:h + 1]

            # --- projN: batched over heads (2 x 512-free MMs per chunk),
            # plain batched evacuation (E2 lives in s_t now)
            projAll = sproj.tile([128, NCHUNK, H * 128], BF16, tag="projAll",
                                 name=f"pa{l}", bufs=1)
            WhV = Wh_ball[l].rearrange("i h f -> i (h f)")
            for c in range(NCHUNK):
                for g in range(2):
                    pN = pmisc.tile([128, N], F32, tag="pbig")
                    nc.tensor.matmul(pN, hT_bf[:, bass.ts(c, 128)],
                                     WhV[:, bass.ts(g, 512)], start=True, stop=True)
                    dst = projAll[:, c, g * 512:(g + 1) * 512]
                    nc.scalar.activation(dst, pN, AF.Copy)

            def proj_ct(h, c):
                return projAll[:, c, h * 128:(h + 1) * 128]

            # --- attention per group of GH heads
            pous = [None] * H
            reps = [None] * H
            for g in range(NG):
                deng_ps = prow.tile([GH, N], F32, tag="deng", bufs=1)
                for j in range(GH):
                    h = g * GH + j
                    tten = sexp.tile([128, NCHUNK, N], BF16, tag="tten", bufs=3)
                    for c in range(NCHUNK):
                        nc.vector.tensor_scalar(tten[:, c, :], e1b[h],
                                                e2_col(h, c), e2a_col(h, c),
                                                OP.mult, OP.max)
                    s_t = sexp.tile([128, NCHUNK, N], BF16, tag="s_t")
                    nc.vector.tensor_tensor(s_t, tten, maskT_all, OP.mult)
                    for c in range(NCHUNK):
                        nc.tensor.matmul(deng_ps, onespad[:, j, :], s_t[:, c, :],
                                         start=(j == 0 and c == 0),
                                         stop=(j == GH - 1 and c == NCHUNK - 1))
                    pou_ps = pou.tile([128, N], F32, tag="oU")
                    for c in range(NCHUNK):
                        nc.tensor.matmul(pou_ps, proj_ct(h, c), s_t[:, c, :],
                                         start=(c == 0), stop=(c == NCHUNK - 1))
                    pob = smulti.tile([128, N], BF16, tag="pob", bufs=8)
                    nc.scalar.activation(pob, pou_ps, AF.Copy)
                    pous[h] = pob
                rr4 = srow.tile([GH, N], F32, tag="rr4")
                nc.vector.reciprocal_approx_fast(rr4, deng_ps)
                rr4_bf = srow.tile([GH, N], BF16, tag="rr4b")
                nc.vector.tensor_copy(rr4_bf, rr4)
                for j in range(GH):
                    h = g * GH + j
                    rep_ps = pmisc.tile([128, N], F32, tag="pbig")
                    nc.tensor.matmul(rep_ps, sel4[:, j, :], rr4_bf,
                                     start=True, stop=True)
                    rp = sbcast.tile([128, N], BF16, tag="rep", bufs=6)
                    nc.scalar.activation(rp, rep_ps, AF.Copy)
                    reps[h] = rp

            # --- normalize + ELU per head
            multiT = []
            for h in range(H):
                outT = sbig.tile([128, N], BF16, tag="outT", bufs=3)
                nc.vector.tensor_tensor(outT, pous[h], reps[h], OP.mult)
                ex = shd.tile([128, N], BF16, tag="elu_ex")
                nc.scalar.activation(ex, outT, AF.Exp)
                ex2 = shd.tile([128, N], BF16, tag="elu_ex2", bufs=3)
                nc.vector.tensor_scalar(ex2, ex, 1.0, -1.0, OP.min, OP.add)
                mh = smulti.tile([128, N], BF16, tag="multi")
                nc.vector.tensor_tensor(mh, outT, ex2, OP.max)
                multiT.append(mh)
                warm(ex)

            # --- W_out projection (h2 in F-layout)
            ph2 = pou.tile([128, N], F32, tag="oU")
            for h in range(H):
                nc.tensor.matmul(ph2, Wo_bf[l][:, h, :], multiT[h],
                                 start=(h == 0), stop=(h == H - 1))
            h2_bf = sbig.tile([128, N], BF16, tag="h2b", bufs=2)
            nc.scalar.activation(h2_bf, ph2, AF.Copy)

            # --- single out-attention
            s12o_ps = prow.tile([2, N], F32, tag="prow")
            nc.tensor.matmul(s12o_ps, ao_bf[l], h2_bf, start=True, stop=True)
            Xo_b = sx.tile([1, N], BF16, tag="Xo_b")    # E1b_o row
            nc.scalar.activation(Xo_b, s12o_ps[0:1, :], AF.Exp, scale=BETA)
            warm(Xo_b)
            ebo_ps = pmisc.tile([128, N], F32, tag="pbig")
            nc.tensor.matmul(ebo_ps, ones_row_bf[:, 0:128], Xo_b,
                             start=True, stop=True)
            e1bo = sbcast.tile([128, N], BF16, tag="e1b", bufs=9)
            nc.scalar.activation(e1bo, ebo_ps, AF.Copy)
            so_ps = prow.tile([128, NCHUNK, 2], F32, tag="prow")
            for c in range(NCHUNK):
                nc.tensor.matmul(so_ps[:, c, :], h2_bf[:, bass.ts(c, 128)],
                                 ao_bf[l], start=True, stop=True)
            Co_e2f = sx.tile([128, NCHUNK, 2], F32, tag="Coe2f")
            nc.scalar.activation(Co_e2f, so_ps, AF.Exp, scale=1.0)
            Co_e2a = sx.tile([128, NCHUNK, 2], F32, tag="Coe2a")
            nc.scalar.activation(Co_e2a, so_ps, AF.Exp, scale=ALPHA)
            # h2N via PE transposes of h2_bf, plain evac (E2_o lives in s_to)
            h2Np = sproj.tile([128, NCHUNK, 128], BF16, tag="h2Np")
            for c in range(NCHUNK):
                pm = pmisc.tile([128, 128], BF16, tag="pbig")
                nc.tensor.transpose(pm, h2_bf[:, bass.ts(c, 128)], ident_bf)
                nc.scalar.activation(h2Np[:, c, :], pm, AF.Copy)
            # out-att scores
            tto = sexp.tile([128, NCHUNK, N], BF16, tag="tten", bufs=3)
            for c in range(NCHUNK):
                nc.vector.tensor_scalar(tto[:, c, :], e1bo,
                                        Co_e2f[:, c, 1:2], Co_e2a[:, c, 1:2],
                                        OP.mult, OP.max)
            s_to = sexp.tile([128, NCHUNK, N], BF16, tag="s_t")
            nc.vector.tensor_tensor(s_to, tto, maskT_all, OP.mult)
            warm(s_to[:, 0, :])
            deno_ps = prow.tile([1, N], F32, tag="prow")
            for c in range(NCHUNK):
                nc.tensor.matmul(deno_ps, ones_col_bf, s_to[:, c, :],
                                 start=(c == 0), stop=(c == NCHUNK - 1))
            pouo_ps = pou.tile([128, N], F32, tag="oU")
            for c in range(NCHUNK):
                nc.tensor.matmul(pouo_ps, h2Np[:, c, :], s_to[:, c, :],
                                 start=(c == 0), stop=(c == NCHUNK - 1))
            rro = srow.tile([1, N], F32, tag="rro")
            nc.vector.reciprocal_approx_fast(rro, deno_ps)
            rro_bf = srow.tile([1, N], BF16, tag="rrob")
            nc.vector.tensor_copy(rro_bf, rro)
            rpo_ps = pmisc.tile([128, N], F32, tag="pbig")
            nc.tensor.matmul(rpo_ps, ones_row_bf[:, 0:128], rro_bf,
                             start=True, stop=True)
            pobo = smulti.tile([128, N], BF16, tag="pob", bufs=8)
            nc.scalar.activation(pobo, pouo_ps, AF.Copy)
            outsT = sbig.tile([128, N], BF16, tag="outsT", bufs=2)
            nc.vector.tensor_tensor(outsT, pobo, rpo_ps, OP.mult)

            # ---- residual + LN over partition dim (bf16 stream) ----
            xs = sbig.tile([128, N], BF16, tag="xs", bufs=2)
            nc.vector.tensor_tensor(xs, outsT, residT, OP.add)
            xsq = sbig.tile([128, N], BF16, tag="xsq", bufs=2)
            nc.scalar.activation(xsq, xs, AF.Square)
            warm(xs)
            # 4-partition stats: row c of [4,128] = mean/meansq of chunk c
            pmu4 = prow.tile([NCHUNK, 128], F32, tag="prow")
            for c in range(NCHUNK):
                nc.tensor.matmul(pmu4, invFpad[:, c, :], xs[:, bass.ts(c, 128)],
                                 start=(c == 0), stop=(c == NCHUNK - 1))
            psq4 = prow.tile([NCHUNK, 128], F32, tag="prow")
            for c in range(NCHUNK):
                nc.tensor.matmul(psq4, invFpad[:, c, :], xsq[:, bass.ts(c, 128)],
                                 start=(c == 0), stop=(c == NCHUNK - 1))
            mu2 = srow.tile([NCHUNK, 128], F32, tag="rowL", bufs=4)
            nc.scalar.activation(mu2, pmu4, AF.Square)
            nmu4_bf = srow.tile([NCHUNK, 128], BF16, tag="rowLb", bufs=2)
            nc.vector.tensor_scalar_mul(nmu4_bf, pmu4, -1.0)
            # rstd = (var+eps)^-1/2 via int32-view seed + 2 Newton steps
            # (avoids Ln/Exp -> no ACT table switching)
            ve = srow.tile([NCHUNK, 128], F32, tag="rowL", bufs=4)
            nc.vector.scalar_tensor_tensor(ve, psq4, LN_EPS, mu2,
                                           OP.add, OP.subtract)
            y0 = srow.tile([NCHUNK, 128], F32, tag="rowL", bufs=4)
            nc.vector.tensor_scalar(y0.bitcast(mybir.dt.int32),
                                    ve.bitcast(mybir.dt.int32),
                                    -0.5, 1597463007.0, OP.mult, OP.add)
            w1 = srow.tile([NCHUNK, 128], F32, tag="rowL", bufs=4)
            nc.vector.tensor_tensor(w1, y0, y0, OP.mult)
            z1 = srow.tile([NCHUNK, 128], F32, tag="rowL", bufs=4)
            nc.vector.scalar_tensor_tensor(z1, ve, -0.5, w1, OP.mult, OP.mult)
            y1 = srow.tile([NCHUNK, 128], F32, tag="rowL", bufs=4)
            nc.vector.scalar_tensor_tensor(y1, z1, 1.5, y0, OP.add, OP.mult)
            w2 = srow.tile([NCHUNK, 128], F32, tag="rowL", bufs=4)
            nc.vector.tensor_tensor(w2, y1, y1, OP.mult)
            z2 = srow.tile([NCHUNK, 128], F32, tag="rowL", bufs=4)
            nc.vector.scalar_tensor_tensor(z2, ve, -0.5, w2, OP.mult, OP.mult)
            rstd4_bf = srow.tile([NCHUNK, 128], BF16, tag="rowLb", bufs=2)
            nc.vector.scalar_tensor_tensor(rstd4_bf, z2, 1.5, y1,
                                           OP.add, OP.mult)
            warm(nmu4_bf)
            rep_rstd = pmisc.tile([128, N], F32, tag="pbig")
            rep_nmu = pmisc.tile([128, N], F32, tag="pbig")
            for c in range(NCHUNK):
                nc.tensor.matmul(rep_rstd[:, bass.ts(c, 128)],
                                 sel4[:, c, :], rstd4_bf,
                                 start=True, stop=True)
                nc.tensor.matmul(rep_nmu[:, bass.ts(c, 128)],
                                 sel4[:, c, :], nmu4_bf,
                                 start=True, stop=True)
            u = sbig.tile([128, N], BF16, tag="u", bufs=2)
            nc.vector.tensor_tensor(u, xs, rep_nmu, OP.add)
            t2 = sbig.tile([128, N], BF16, tag="t2", bufs=2)
            nc.vector.tensor_tensor(t2, u, rep_rstd, OP.mult)
            hT_bf = sbig.tile([128, N], BF16, tag="hTb", bufs=2)
            fn = AF.Relu if l < L - 1 else AF.Identity
            nc.scalar.activation(hT_bf, t2, fn, scale=g_col[l], bias=b_col[l])
            hT = hT_bf

        # ---------------- output: transpose back ----------------
        for c in range(NCHUNK):
            po = pmisc.tile([128, 128], BF16, tag="pbig")
            nc.tensor.transpose(po, hT[:, bass.ts(c, 128)], ident_bf)
            osb = shd.tile([128, 128], F32, tag="osb")
            nc.scalar.activation(osb, po, AF.Copy)
            nc.sync.dma_start(out_d[bass.ts(c, 128), :], osb)

    nc.compile()
    return nc


def _get_nc():
    if "nc" not in _CACHE:
        _CACHE["nc"] = build_nc()
    return _CACHE["nc"]


def kernel(**inputs) -> np.ndarray:
    nc = _get_nc()
    shared = {k: np.ascontiguousarray(np.asarray(inputs[k], dtype=np.float32))
              for k in ("Wp", "bp", "W_heads", "a_heads", "W_out", "a_out",
                        "ln_g", "ln_b")}
    x = np.asarray(inputs["x"], dtype=np.float32)
    adj = np.asarray(inputs["adj"], dtype=np.int32)
    in_maps = [dict(x=np.ascontiguousarray(x[b]),
                    adj=np.ascontiguousarray(adj[b]), **shared)
               for b in range(B)]
    res = run_bass_kernel_spmd(nc, in_maps, core_ids=list(range(B)))
    return np.stack([res.results[b]["out"] for b in range(B)])


if __name__ == "__main__":
    rng = np.random.default_rng(0)
    inputs = dict(
        x=rng.normal(size=(B, N, DIN)).astype(np.float32),
        adj=rng.integers(0, 2, size=(B, N, N)).astype(np.int32),
        Wp=(rng.normal(size=(DIN, F)) * 0.12).astype(np.float32),
        bp=np.zeros(F, dtype=np.float32),
        W_heads=(rng.normal(size=(L, H, F, F)) * 0.08).astype(np.float32),
        a_heads=(rng.normal(size=(L, H, 2 * F)) * 0.08).astype(np.float32),
        W_out=(rng.normal(size=(L, H * F, F)) * 0.03).astype(np.float32),
        a_out=(rng.normal(size=(L, 2 * F)) * 0.08).astype(np.float32),
        ln_g=np.ones((L, F), dtype=np.float32),
        ln_b=np.zeros((L, F), dtype=np.float32),
    )
    out = kernel(**inputs)
    print("out", out.shape, out.dtype, np.abs(out).max())
